# revision 11
# baseline (speedup 1.0000x reference)
"""Trainium2 Bass kernel for nn_MultiHeadAttention_6786048328624 (sparse_attention).

Strategy (8 NeuronCores, data-parallel over batch B=8, one batch per core):

Math restructure (exactly equivalent to the reference in fp32, verified):
  - scores are computed TRANSPOSED per head: S^T[k,q] = Kh @ Qh^T, so that the
    attention-weighted V contraction (over k) needs no on-chip transposes:
    out_h^T[dk,q] = [Vh | 1]^T @ attn^T, where the appended ones-column yields
    the softmax denominator Z[q] for free in psum row 64.
  - softmax skips the max-subtraction: scores/8 + bias is bounded (|x| <~ 5),
    exp() is exact-safe in fp32/fp16 range. Verified vs reference: rel ~ 3e-6
    in fp32, ~6e-4 with the fp16 hot path used here.
  - mask is folded additively into the bias: logb = w0*f(t) + w1*f(d) + b_bias
    + (mask-1)*50;  exp(logb) == 0 (fp16 underflow) where masked, which matches
    the reference's -1e9 masking to well below float resolution.
  - bias mats broadcast over heads: eb = exp(logb) is computed once per batch
    and multiplied into exp(scores) per head (exp(s+b) = exp(s)*exp(b)).
  - k-projection bias bk provably cancels in softmax (constant along the
    softmax axis); v/out biases fold into a host-side constant row added after
    gather (all zero in this problem's setup_inputs); bq must be zero.

Precision: all matmuls fp16 with fp32 PSUM accumulation; softmax denominator Z
and its reciprocal in fp32 (broadcast to 64 partitions via a DRAM-bounce DMA).
End-to-end rel err vs fp32 reference ~6e-4.

Layouts: host pre-transposes q/k/v to [D,S] and temporal/dis/mask to [k,q]
(pure relayout during sharding; same bytes DMA'd). Weights are replicated
per-core and shipped pre-converted to fp16. All device DMAs are large
contiguous blocks.

Engine assignment is balanced against the instruction cost model:
ACT: Ln/exp (transcendentals) + some evacs; DVE: fp16 2x/4x elementwise,
psum evacs; GPSIMD: mask convert, part of the attn multiply; PE: matmuls.
"""

import numpy as np
from contextlib import ExitStack

import concourse.bass as bass
import concourse.tile as tile
from concourse import bacc, mybir
from concourse.bass_utils import run_bass_kernel_spmd

F32 = mybir.dt.float32
F16 = mybir.dt.float16
I32 = mybir.dt.int32

B, S, D, H, DK = 8, 1024, 512, 8, 64
NT = S // 128        # 8 row tiles of 128
NC = D // 128        # 4 chunks of the model dim
MASK_NEG = 50.0


def build_nc(mul_gpsimd_kts=(5, 6, 7), reps=1):
    """Build the per-core Bass program (SPMD; every core runs one batch).

    reps>1 wraps the entire body in a hardware For_i loop — used only for
    benchmarking (amortizes host->device dispatch overhead across reps)."""
    import contextlib
    nc = bacc.Bacc("TRN2", target_bir_lowering=False, debug=False)

    qT_d = nc.dram_tensor("qT", [D, S], F32, kind="ExternalInput").ap()
    kT_d = nc.dram_tensor("kT", [D, S], F32, kind="ExternalInput").ap()
    vT_d = nc.dram_tensor("vT", [D, S], F32, kind="ExternalInput").ap()
    tT_d = nc.dram_tensor("tT", [S, S], F32, kind="ExternalInput").ap()
    dT_d = nc.dram_tensor("dT", [S, S], F32, kind="ExternalInput").ap()
    mT_d = nc.dram_tensor("mT", [S, S], I32, kind="ExternalInput").ap()
    wq_d = nc.dram_tensor("Wq16", [D, D], F16, kind="ExternalInput").ap()
    wk_d = nc.dram_tensor("Wk16", [D, D], F16, kind="ExternalInput").ap()
    wv_d = nc.dram_tensor("Wv16", [D, D], F16, kind="ExternalInput").ap()
    wo_d = nc.dram_tensor("Wo16", [D, D], F16, kind="ExternalInput").ap()
    consts_d = nc.dram_tensor("consts", [4], F32, kind="ExternalInput").ap()
    out_d = nc.dram_tensor("out", [S, D], F32, kind="ExternalOutput").ap()

    def bcast128(dram_ap, j):
        return bass.AP(tensor=dram_ap.tensor, offset=dram_ap.offset + j,
                       ap=[[0, 128], [1, 1]])

    with tile.TileContext(nc) as tc, ExitStack() as ctx:
        ctx.enter_context(nc.allow_low_precision(
            reason="fp16 hot path validated vs fp32 reference (rel ~6e-4)"))
        persist = ctx.enter_context(tc.tile_pool(name="persist", bufs=1))
        xload = ctx.enter_context(tc.tile_pool(name="xload", bufs=4))
        bload = ctx.enter_context(tc.tile_pool(name="bload", bufs=2))
        bwork = ctx.enter_context(tc.tile_pool(name="bwork", bufs=2))
        espool = ctx.enter_context(tc.tile_pool(name="espool", bufs=3))
        zpool = ctx.enter_context(tc.tile_pool(name="zpool", bufs=2))
        outsb = ctx.enter_context(tc.tile_pool(name="outsb", bufs=2))
        ps_s = ctx.enter_context(tc.tile_pool(name="ps_s", bufs=2, space="PSUM"))
        ps_o = ctx.enter_context(tc.tile_pool(name="ps_o", bufs=2, space="PSUM"))
        zdram = ctx.enter_context(tc.tile_pool(name="zdram", bufs=2, space="DRAM"))

        if reps > 1:
            ctx.enter_context(tc.For_i(
                0, reps, 1,
                hint_engines=(mybir.EngineType.PE, mybir.EngineType.Activation,
                              mybir.EngineType.DVE, mybir.EngineType.Pool,
                              mybir.EngineType.SP)))

        # ---- scalars ----
        w0b = persist.tile([128, 1], F32, tag="w0b")
        w1b = persist.tile([128, 1], F32, tag="w1b")
        bbm = persist.tile([128, 1], F32, tag="bbm")
        nc.sync.dma_start(w0b[:], bcast128(consts_d, 0))
        nc.sync.dma_start(w1b[:], bcast128(consts_d, 1))
        nc.sync.dma_start(bbm[:], bcast128(consts_d, 2))
        e_t = persist.tile([128, 1], F32, tag="e_t")
        nc.vector.memset(e_t[:], float(np.e))

        # ---- weights (already fp16 in DRAM) ----
        def load_w(dram, name, parts, rows):
            tiles = []
            for c in range(rows):
                w16 = persist.tile([parts, D], F16, tag=f"{name}{c}")
                nc.sync.dma_start(w16[:], dram[c * parts:(c + 1) * parts, :])
                tiles.append(w16)
            return tiles

        wq16 = load_w(wq_d, "wq", 128, NC)
        wk16 = load_w(wk_d, "wk", 128, NC)
        wv16 = load_w(wv_d, "wv", 128, NC)
        wo16 = load_w(wo_d, "wo", 128, NC)     # [128,512] head-pair chunks

        # ---- q/k/v loads + fp16 conversion (DVE 2x) ----
        def load_x16(dram):
            xs = []
            for kc in range(NC):
                xf = xload.tile([128, S], F32, tag="xf")
                nc.sync.dma_start(xf[:], dram[kc * 128:(kc + 1) * 128, :])
                x16 = xload.tile([128, S], F16, tag="x16")
                nc.vector.tensor_copy(x16[:], xf[:])
                xs.append(x16)
            return xs

        xq = load_x16(qT_d)
        xk = load_x16(kT_d)
        xv = load_x16(vT_d)

        # ---- fused bias tiles (independent of projections; ACT-heavy,
        #      scheduled alongside the PE projection phase) ----
        EB = []
        for kt in range(NT):
            tld = bload.tile([128, S], F32, tag="tld")
            nc.sync.dma_start(tld[:], tT_d[kt * 128:(kt + 1) * 128, :])
            dld = bload.tile([128, S], F32, tag="dld")
            nc.sync.dma_start(dld[:], dT_d[kt * 128:(kt + 1) * 128, :])
            mld = bload.tile([128, S], I32, tag="mld")
            nc.sync.dma_start(mld[:], mT_d[kt * 128:(kt + 1) * 128, :])

            L1 = bwork.tile([128, S], F16, tag="L1")
            nc.scalar.activation(L1[:], tld[:], mybir.ActivationFunctionType.Ln,
                                 bias=e_t[:], scale=100.0)
            nc.vector.reciprocal(L1[:], L1[:])
            L2 = bwork.tile([128, S], F16, tag="L2")
            nc.scalar.activation(L2[:], dld[:], mybir.ActivationFunctionType.Ln,
                                 bias=e_t[:], scale=100.0)
            nc.vector.reciprocal(L2[:], L2[:])
            mterm = bwork.tile([128, S], F16, tag="mterm")
            nc.gpsimd.tensor_scalar(mterm[:], mld[:], MASK_NEG, -MASK_NEG,
                                    mybir.AluOpType.mult, mybir.AluOpType.add)
            nc.vector.scalar_tensor_tensor(L1[:], L1[:], w0b[:], mterm[:],
                                           mybir.AluOpType.mult,
                                           mybir.AluOpType.add)
            nc.vector.scalar_tensor_tensor(L1[:], L2[:], w1b[:], L1[:],
                                           mybir.AluOpType.mult,
                                           mybir.AluOpType.add)
            eb = persist.tile([128, S], F16, tag=f"eb{kt}")
            # b_bias enters via the ACT per-partition bias operand
            nc.scalar.activation(eb[:], L1[:], mybir.ActivationFunctionType.Exp,
                                 bias=bbm[:])
            EB.append(eb)

        # ---- projections ----
        # QT/KT[d,s]: lhsT = W chunk [din,dout], rhs = X^T [din,s]
        QT16, KT16 = [], []
        for w16, xs, name, dst in [(wq16, xq, "qt", QT16),
                                   (wk16, xk, "kt", KT16)]:
            for c in range(NC):
                ps = ps_s.tile([128, S], F32, tag="sT")
                for kc in range(NC):
                    for j in range(2):
                        nc.tensor.matmul(
                            ps[:, j * 512:(j + 1) * 512],
                            w16[kc][:, c * 128:(c + 1) * 128],
                            xs[kc][:, j * 512:(j + 1) * 512],
                            start=(kc == 0), stop=(kc == NC - 1),
                            skip_group_check=True)
                t16 = persist.tile([128, S], F16, tag=f"{name}{c}")
                nc.scalar.copy(t16[:], ps[:])
                dst.append(t16)

        # V[s,d]: lhsT = X^T chunk [din, stile], rhs = Wv chunk [din, dout];
        # evac into [128, 8, 65] per s-tile with a ones column per head
        V_sb = []
        for st in range(NT):
            ps = ps_o.tile([128, D], F32, tag="ot")
            for kc in range(NC):
                nc.tensor.matmul(ps[:], xv[kc][:, st * 128:(st + 1) * 128],
                                 wv16[kc][:], start=(kc == 0),
                                 stop=(kc == NC - 1), skip_group_check=True)
            vt = persist.tile([128, H, 65], F16, tag=f"v{st}")
            nc.scalar.copy(vt[:, :, 0:64], ps.rearrange("p (h d) -> p h d", h=H))
            nc.gpsimd.memset(vt[:, :, 64:65], 1.0)
            V_sb.append(vt)

        # ---- attention heads ----
        # OutT pair tiles [128, S]: even head -> rows 0:64 (direct),
        # odd head -> staging tile then SBUF->SBUF DMA into rows 64:128.
        OutP = [persist.tile([128, S], F16, tag=f"op{p}", name=f"op{p}")
                for p in range(H // 2)]
        for h in range(H):
            c, hh = h // 2, h % 2
            qh = QT16[c][hh * 64:(hh + 1) * 64, :]
            ot = ps_o.tile([65, S], F32, tag="ot")
            for kt in range(NT):
                sps = ps_s.tile([128, S], F32, tag="sT")
                kh = KT16[c][hh * 64:(hh + 1) * 64, kt * 128:(kt + 1) * 128]
                for j in range(2):
                    nc.tensor.matmul(sps[:, j * 512:(j + 1) * 512], kh,
                                     qh[:, j * 512:(j + 1) * 512],
                                     start=True, stop=True,
                                     skip_group_check=True)
                es = espool.tile([128, S], F16, tag="es")
                nc.scalar.activation(es[:], sps[:],
                                     mybir.ActivationFunctionType.Exp,
                                     scale=1.0 / 8.0)
                at = espool.tile([128, S], F16, tag="at")
                eng = nc.gpsimd if kt in mul_gpsimd_kts else nc.vector
                eng.tensor_tensor(at[:], es[:], EB[kt][:],
                                  op=mybir.AluOpType.mult)
                for j in range(2):
                    nc.tensor.matmul(ot[:, j * 512:(j + 1) * 512],
                                     V_sb[kt][:, h, :],
                                     at[:, j * 512:(j + 1) * 512],
                                     start=(kt == 0), stop=(kt == NT - 1),
                                     skip_group_check=True)
            # Z = ot row 64 -> recip (fp32, psum source) -> DRAM bounce -> bcast
            ztmp = zpool.tile([65, S], F32, tag="ztmp")
            nc.vector.reciprocal(ztmp[64:65, :], ot[64:65, :])
            zd = zdram.tile([1, S], F32, tag="zd")
            nc.sync.dma_start(zd[:], ztmp[64:65, :])
            zb = zpool.tile([64, S], F32, tag="zb")
            nc.sync.dma_start(zb[:], bass.AP(tensor=zd.tensor, offset=zd.offset,
                                             ap=[[0, 64], [1, S]]))
            if hh == 0:
                nc.vector.tensor_tensor(OutP[c][0:64, :], ot[0:64, :], zb[:],
                                        op=mybir.AluOpType.mult)
            else:
                o16 = zpool.tile([64, S], F16, tag="o16")
                nc.vector.tensor_tensor(o16[:], ot[0:64, :], zb[:],
                                        op=mybir.AluOpType.mult)
                nc.sync.dma_start(OutP[c][64:128, :], o16[:])

        # ---- output projection: K=128 per head-pair ----
        for st in range(NT):
            f = ps_o.tile([128, D], F32, tag="ot")
            for p in range(H // 2):
                nc.tensor.matmul(f[:], OutP[p][:, st * 128:(st + 1) * 128],
                                 wo16[p][:], start=(p == 0),
                                 stop=(p == H // 2 - 1), skip_group_check=True)
            o = outsb.tile([128, D], F32, tag="o")
            nc.scalar.copy(o[:], f[:])
            nc.sync.dma_start(out_d[st * 128:(st + 1) * 128, :], o[:])

    nc.compile()
    return nc


_NC = None


def make_in_maps(q, k, v, temporal_mat, dis_mat, mask, Wq, Wk, Wv, Wo,
                 w_bias, b_bias):
    consts = np.array([w_bias[0], w_bias[1], float(b_bias), 0.0], np.float32)
    in_maps = []
    for b in range(B):
        in_maps.append({
            "qT": np.ascontiguousarray(q[b].T),
            "kT": np.ascontiguousarray(k[b].T),
            "vT": np.ascontiguousarray(v[b].T),
            "tT": np.ascontiguousarray(temporal_mat[b].T),
            "dT": np.ascontiguousarray(dis_mat[b].T),
            "mT": np.ascontiguousarray(mask[b].T),
            "Wq16": Wq.astype(np.float16), "Wk16": Wk.astype(np.float16),
            "Wv16": Wv.astype(np.float16), "Wo16": Wo.astype(np.float16),
            "consts": consts,
        })
    return in_maps


def kernel(q, k, v, temporal_mat, dis_mat, mask,
           Wq, bq, Wk, bk, Wv, bv, w_bias, b_bias, Wo, bo):
    global _NC
    q = np.asarray(q, np.float32)
    k = np.asarray(k, np.float32)
    v = np.asarray(v, np.float32)
    temporal_mat = np.asarray(temporal_mat, np.float32)
    dis_mat = np.asarray(dis_mat, np.float32)
    mask = np.asarray(mask, np.int32)
    Wq, Wk, Wv, Wo = (np.asarray(x, np.float32) for x in (Wq, Wk, Wv, Wo))
    w_bias = np.asarray(w_bias, np.float32)
    b_bias = np.asarray(b_bias, np.float32).reshape(())

    # bk cancels exactly in softmax; bv/bo fold into a constant output row
    # added after the gather; bq would change scores (must be zero here).
    assert np.allclose(np.asarray(bq), 0.0), "nonzero bq unsupported"
    bo_eff = np.asarray(bv, np.float32) @ Wo + np.asarray(bo, np.float32)

    if _NC is None:
        _NC = build_nc()

    in_maps = make_in_maps(q, k, v, temporal_mat, dis_mat, mask,
                           Wq, Wk, Wv, Wo, w_bias, b_bias)
    res = run_bass_kernel_spmd(_NC, in_maps, core_ids=list(range(B)))
    out = np.stack([r["out"] for r in res.results], axis=0)
    if np.any(bo_eff != 0.0):
        out = out + bo_eff[None, None, :]
    return out.astype(np.float32)


# revision 25
# speedup vs baseline: 1.3261x; 1.3261x over previous
"""Trainium2 Bass kernel for nn_MultiHeadAttention_6786048328624 (sparse_attention).

Strategy (8 NeuronCores, data-parallel over batch B=8, one batch per core):

Math restructure (exactly equivalent to the reference in fp32, verified):
  - scores are computed TRANSPOSED per head: S^T[k,q] = Kh @ Qh^T, so that the
    attention-weighted V contraction (over k) needs no on-chip transposes:
    out_h^T[dk,q] = [Vh | 1]^T @ attn^T, where the appended ones-column yields
    the softmax denominator Z[q] for free in psum row 64.
  - softmax skips the max-subtraction: scores/8 + bias is bounded (|x| <~ 5),
    exp() is exact-safe in fp32/fp16 range. Verified vs reference: rel ~ 3e-6
    in fp32, ~6e-4 with the fp16 hot path used here.
  - mask is folded additively into the bias: logb = w0*f(t) + w1*f(d) + b_bias
    + (mask-1)*50;  exp(logb) == 0 (fp16 underflow) where masked, which matches
    the reference's -1e9 masking to well below float resolution.
  - bias mats broadcast over heads: eb = exp(logb) is computed once per batch
    and multiplied into exp(scores) per head (exp(s+b) = exp(s)*exp(b)).
  - k-projection bias bk provably cancels in softmax (constant along the
    softmax axis); v/out biases fold into a host-side constant row added after
    gather (all zero in this problem's setup_inputs); bq must be zero.

Precision: all matmuls fp16 with fp32 PSUM accumulation; softmax denominator Z
and its reciprocal in fp32 (broadcast to 64 partitions via a DRAM-bounce DMA).
End-to-end rel err vs fp32 reference ~6e-4.

Layouts: host pre-transposes q/k/v to [D,S] and temporal/dis/mask to [k,q]
(pure relayout during sharding; same bytes DMA'd). Weights are replicated
per-core and shipped pre-converted to fp16. All device DMAs are large
contiguous blocks.

Engine assignment notes: ACT runs ONLY Ln/Exp (activation-table switches cost
~1.5us, so no Copy evacs on ACT, and Lns are grouped before Exps); DVE takes
fp16 2x elementwise + all psum evacuations; GPSIMD takes mask convert, the
scalar_tensor_tensor combines (w0/w1 baked as immediates) and part of the
attention multiply; PE does fp16 matmuls only.
"""

import numpy as np
from contextlib import ExitStack

import concourse.bass as bass
import concourse.tile as tile
from concourse import bacc, mybir
from concourse.bass_utils import run_bass_kernel_spmd

F32 = mybir.dt.float32
F16 = mybir.dt.float16
I32 = mybir.dt.int32
AF = mybir.ActivationFunctionType
ALU = mybir.AluOpType

B, S, D, H, DK = 8, 1024, 512, 8, 64
NT = S // 128        # 8 row tiles of 128
NC = D // 128        # 4 chunks of the model dim
MASK_NEG = 50.0


def build_nc(w0=0.0, w1=0.0, bb=0.0, mul_gpsimd_kts=(5, 6, 7), reps=1):
    """Build the per-core Bass program (SPMD; every core runs one batch).

    w0/w1/bb are the (scalar) Linear(2,1) bias-branch weights, baked as
    immediates. reps>1 wraps the body in a hardware For_i loop (bench only).
    """
    nc = bacc.Bacc("TRN2", target_bir_lowering=False, debug=False)

    qT_d = nc.dram_tensor("qT", [D, S], F32, kind="ExternalInput").ap()
    kT_d = nc.dram_tensor("kT", [D, S], F32, kind="ExternalInput").ap()
    vT_d = nc.dram_tensor("vT", [D, S], F32, kind="ExternalInput").ap()
    tT_d = nc.dram_tensor("tT", [S, S], F32, kind="ExternalInput").ap()
    dT_d = nc.dram_tensor("dT", [S, S], F32, kind="ExternalInput").ap()
    mT_d = nc.dram_tensor("mT", [S, S], I32, kind="ExternalInput").ap()
    wq_d = nc.dram_tensor("Wq16", [D, D], F16, kind="ExternalInput").ap()
    wk_d = nc.dram_tensor("Wk16", [D, D], F16, kind="ExternalInput").ap()
    wv_d = nc.dram_tensor("Wv16", [D, D], F16, kind="ExternalInput").ap()
    wo_d = nc.dram_tensor("Wo16", [D, D], F16, kind="ExternalInput").ap()
    out_d = nc.dram_tensor("out", [S, D], F32, kind="ExternalOutput").ap()

    with tile.TileContext(nc) as tc, ExitStack() as ctx:
        ctx.enter_context(nc.allow_low_precision(
            reason="fp16 hot path validated vs fp32 reference (rel ~6e-4)"))
        persist = ctx.enter_context(tc.tile_pool(name="persist", bufs=1))
        xload = ctx.enter_context(tc.tile_pool(name="xload", bufs=4))
        bload = ctx.enter_context(tc.tile_pool(name="bload", bufs=2))
        bwork = ctx.enter_context(tc.tile_pool(name="bwork", bufs=1))
        espool = ctx.enter_context(tc.tile_pool(name="espool", bufs=2))
        zpool = ctx.enter_context(tc.tile_pool(name="zpool", bufs=2))
        outsb = ctx.enter_context(tc.tile_pool(name="outsb", bufs=2))
        ps_s = ctx.enter_context(tc.tile_pool(name="ps_s", bufs=2, space="PSUM"))
        ps_o = ctx.enter_context(tc.tile_pool(name="ps_o", bufs=2, space="PSUM"))
        zdram = ctx.enter_context(tc.tile_pool(name="zdram", bufs=2, space="DRAM"))

        if reps > 1:
            ctx.enter_context(tc.For_i(
                0, reps, 1,
                hint_engines=(mybir.EngineType.PE, mybir.EngineType.Activation,
                              mybir.EngineType.DVE, mybir.EngineType.Pool,
                              mybir.EngineType.SP)))

        e_t = persist.tile([128, 1], F32, tag="e_t")
        nc.vector.memset(e_t[:], float(np.e))

        # ---- weights (already fp16 in DRAM) ----
        def load_w(dram, name):
            tiles = []
            for c in range(NC):
                w16 = persist.tile([128, D], F16, tag=f"{name}{c}",
                                   name=f"{name}{c}")
                nc.sync.dma_start(w16[:], dram[c * 128:(c + 1) * 128, :])
                tiles.append(w16)
            return tiles

        wq16 = load_w(wq_d, "wq")
        wk16 = load_w(wk_d, "wk")
        wv16 = load_w(wv_d, "wv")
        wo16 = load_w(wo_d, "wo")     # [128,512] head-pair chunks

        # ---- q/k/v loads + fp16 conversion (GPSIMD: 1-input ops are cheap) ----
        def load_x16(dram):
            xs = []
            for kc in range(NC):
                xf = xload.tile([128, S], F32, tag="xf", bufs=2)
                nc.sync.dma_start(xf[:], dram[kc * 128:(kc + 1) * 128, :])
                x16 = xload.tile([128, S], F16, tag="x16")
                nc.gpsimd.tensor_copy(x16[:], xf[:])
                xs.append(x16)
            return xs

        xq = load_x16(qT_d)
        xk = load_x16(kT_d)
        xv = load_x16(vT_d)

        # ---- fused bias, in blocks of 4 k-tiles: Lns grouped, then the DVE
        #      combine chain, then Exps — keeps ACT table switches rare ----
        lpool = ctx.enter_context(tc.tile_pool(name="lpool", bufs=1))
        EB = []
        for blk in range(0, NT, 4):
            Ls, Ms = [], []
            for kt in range(blk, blk + 4):
                tld = bload.tile([128, S], F32, tag="tld")
                nc.sync.dma_start(tld[:], tT_d[kt * 128:(kt + 1) * 128, :])
                L1 = lpool.tile([128, S], F32, tag=f"L1_{kt % 4}",
                                name=f"L1_{kt % 4}")
                nc.scalar.activation(L1[:], tld[:], AF.Ln, bias=e_t[:],
                                     scale=100.0)
                dld = bload.tile([128, S], F32, tag="dld")
                nc.sync.dma_start(dld[:], dT_d[kt * 128:(kt + 1) * 128, :])
                L2 = lpool.tile([128, S], F32, tag=f"L2_{kt % 4}",
                                name=f"L2_{kt % 4}")
                nc.scalar.activation(L2[:], dld[:], AF.Ln, bias=e_t[:],
                                     scale=100.0)
                Ls.append((L1, L2))
                mld = bload.tile([128, S], I32, tag="mld")
                nc.sync.dma_start(mld[:], mT_d[kt * 128:(kt + 1) * 128, :])
                mterm = bwork.tile([128, S], F32, tag=f"mterm{kt % 4}",
                                   name=f"mterm{kt % 4}")
                nc.gpsimd.tensor_scalar(mterm[:], mld[:], MASK_NEG,
                                        bb - MASK_NEG, ALU.mult, ALU.add)
                Ms.append(mterm)
            for i, kt in enumerate(range(blk, blk + 4)):
                L1, L2 = Ls[i]
                # recip_approx is multi-pass: no in-place aliasing
                R1 = bwork.tile([128, S], F32, tag="R1", bufs=2)
                nc.vector.reciprocal_approx_fast(R1[:], L1[:])
                R2 = bwork.tile([128, S], F32, tag="R2", bufs=2)
                nc.vector.reciprocal_approx_fast(R2[:], L2[:])
                nc.vector.scalar_tensor_tensor(R1[:], R1[:], w0, Ms[i][:],
                                               ALU.mult, ALU.add)
                nc.vector.scalar_tensor_tensor(R2[:], R2[:], w1, R1[:],
                                               ALU.mult, ALU.add)
                eb = persist.tile([128, S], F16, tag=f"eb{kt}", name=f"eb{kt}")
                nc.scalar.activation(eb[:], R2[:], AF.Exp)
                EB.append(eb)

        # ---- projections ----
        QT16, KT16 = [], []
        for w16, xs, name, dst in [(wq16, xq, "qt", QT16),
                                   (wk16, xk, "kt", KT16)]:
            for c in range(NC):
                ps = ps_s.tile([128, S], F32, tag="sT")
                for kc in range(NC):
                    for j in range(2):
                        nc.tensor.matmul(
                            ps[:, j * 512:(j + 1) * 512],
                            w16[kc][:, c * 128:(c + 1) * 128],
                            xs[kc][:, j * 512:(j + 1) * 512],
                            start=(kc == 0), stop=(kc == NC - 1),
                            skip_group_check=True)
                t16 = persist.tile([128, S], F16, tag=f"{name}{c}",
                                   name=f"{name}{c}")
                nc.vector.tensor_copy(t16[:], ps[:])
                dst.append(t16)

        V_sb = []
        for st in range(NT):
            ps = ps_o.tile([128, D], F32, tag="ot")
            for kc in range(NC):
                nc.tensor.matmul(ps[:], xv[kc][:, st * 128:(st + 1) * 128],
                                 wv16[kc][:], start=(kc == 0),
                                 stop=(kc == NC - 1), skip_group_check=True)
            vt = persist.tile([128, H, 65], F16, tag=f"v{st}", name=f"v{st}")
            nc.vector.tensor_copy(
                vt[:, :, 0:64], ps.rearrange("p (h d) -> p h d", h=H))
            nc.gpsimd.memset(vt[:, :, 64:65], 1.0)
            V_sb.append(vt)

        # ---- attention heads ----
        OutP = [persist.tile([128, S], F16, tag=f"op{p}", name=f"op{p}")
                for p in range(H // 2)]
        for h in range(H):
            c, hh = h // 2, h % 2
            qh = QT16[c][hh * 64:(hh + 1) * 64, :]
            ot = ps_o.tile([65, S], F32, tag="ot")
            for kt in range(NT):
                sps = ps_s.tile([128, S], F32, tag="sT")
                kh = KT16[c][hh * 64:(hh + 1) * 64, kt * 128:(kt + 1) * 128]
                for j in range(2):
                    nc.tensor.matmul(sps[:, j * 512:(j + 1) * 512], kh,
                                     qh[:, j * 512:(j + 1) * 512],
                                     start=True, stop=True,
                                     skip_group_check=True)
                es = espool.tile([128, S], F16, tag="es")
                nc.scalar.activation(es[:], sps[:], AF.Exp, scale=1.0 / 8.0)
                at = espool.tile([128, S], F16, tag="at")
                eng = nc.gpsimd if kt in mul_gpsimd_kts else nc.vector
                eng.tensor_tensor(at[:], es[:], EB[kt][:], op=ALU.mult)
                for j in range(2):
                    nc.tensor.matmul(ot[:, j * 512:(j + 1) * 512],
                                     V_sb[kt][:, h, :],
                                     at[:, j * 512:(j + 1) * 512],
                                     start=(kt == 0), stop=(kt == NT - 1),
                                     skip_group_check=True)
            # Z = ot row 64 -> sbuf -> DRAM bounce broadcast -> recip -> norm
            ztmp = zpool.tile([65, S], F32, tag="ztmp", bufs=1)
            nc.vector.tensor_copy(ztmp[64:65, :], ot[64:65, :])
            zd = zdram.tile([1, S], F32, tag="zd")
            nc.sync.dma_start(zd[:], ztmp[64:65, :])
            zb = zpool.tile([64, S], F32, tag="zb")
            nc.sync.dma_start(zb[:], bass.AP(tensor=zd.tensor, offset=zd.offset,
                                             ap=[[0, 64], [1, S]]))
            zbr = zpool.tile([64, S], F32, tag="zbr")
            nc.vector.reciprocal_approx_fast(zbr[:], zb[:])
            if hh == 0:
                nc.vector.tensor_tensor(OutP[c][0:64, :], ot[0:64, :], zbr[:],
                                        op=ALU.mult)
            else:
                o16 = zpool.tile([64, S], F16, tag="o16")
                nc.vector.tensor_tensor(o16[:], ot[0:64, :], zbr[:],
                                        op=ALU.mult)
                nc.sync.dma_start(OutP[c][64:128, :], o16[:])

        # ---- output projection: K=128 per head-pair ----
        for st in range(NT):
            f = ps_o.tile([128, D], F32, tag="ot")
            for p in range(H // 2):
                nc.tensor.matmul(f[:], OutP[p][:, st * 128:(st + 1) * 128],
                                 wo16[p][:], start=(p == 0),
                                 stop=(p == H // 2 - 1), skip_group_check=True)
            o = outsb.tile([128, D], F32, tag="o")
            nc.scalar.copy(o[:], f[:])
            nc.sync.dma_start(out_d[st * 128:(st + 1) * 128, :], o[:])

    nc.compile()
    return nc


_NC = None


def make_in_maps(q, k, v, temporal_mat, dis_mat, mask, Wq, Wk, Wv, Wo,
                 w_bias=None, b_bias=None):
    in_maps = []
    for b in range(B):
        in_maps.append({
            "qT": np.ascontiguousarray(q[b].T),
            "kT": np.ascontiguousarray(k[b].T),
            "vT": np.ascontiguousarray(v[b].T),
            "tT": np.ascontiguousarray(temporal_mat[b].T),
            "dT": np.ascontiguousarray(dis_mat[b].T),
            "mT": np.ascontiguousarray(mask[b].T),
            "Wq16": Wq.astype(np.float16), "Wk16": Wk.astype(np.float16),
            "Wv16": Wv.astype(np.float16), "Wo16": Wo.astype(np.float16),
        })
    return in_maps


def kernel(q, k, v, temporal_mat, dis_mat, mask,
           Wq, bq, Wk, bk, Wv, bv, w_bias, b_bias, Wo, bo):
    global _NC
    q = np.asarray(q, np.float32)
    k = np.asarray(k, np.float32)
    v = np.asarray(v, np.float32)
    temporal_mat = np.asarray(temporal_mat, np.float32)
    dis_mat = np.asarray(dis_mat, np.float32)
    mask = np.asarray(mask, np.int32)
    Wq, Wk, Wv, Wo = (np.asarray(x, np.float32) for x in (Wq, Wk, Wv, Wo))
    w_bias = np.asarray(w_bias, np.float32)
    b_bias = float(np.asarray(b_bias, np.float32).reshape(()))

    # bk cancels exactly in softmax; bv/bo fold into a constant output row
    # added after the gather; bq would change scores (must be zero here).
    assert np.allclose(np.asarray(bq), 0.0), "nonzero bq unsupported"
    bo_eff = np.asarray(bv, np.float32) @ Wo + np.asarray(bo, np.float32)

    if _NC is None:
        _NC = build_nc(float(w_bias[0]), float(w_bias[1]), b_bias)

    in_maps = make_in_maps(q, k, v, temporal_mat, dis_mat, mask,
                           Wq, Wk, Wv, Wo)
    res = run_bass_kernel_spmd(_NC, in_maps, core_ids=list(range(B)))
    out = np.stack([r["out"] for r in res.results], axis=0)
    if np.any(bo_eff != 0.0):
        out = out + bo_eff[None, None, :]
    return out.astype(np.float32)
